# revision 4
# baseline (speedup 1.0000x reference)
"""Per-batch brute-force 1-NN (nearest cluster) on 8 Trainium2 cores.

Problem: coords1 [L1=4096, N=8, C=3] (reference points), coords2 [L2=4096, N=8, C=3]
(query points). For each batch n and query l, find argmin_m ||q - r||^2 within the
batch. Output: (clusters [L2*N] int32, batch_idx [L2*N] int32), matching
   nearest = argmin(d2, axis=-1) [N, L2]; clusters = nearest.T.reshape(-1)
   batch_idx = broadcast(arange(N), (L2, N)).reshape(-1)

Sharding: batch n -> core n (data parallel, no cross-core communication).

Device kernel (per core, one batch):
  - Host packs qrT [4, L2+L1] fp32: cols 0..L2-1 = (qx,qy,qz,1) per query,
    cols L2.. = (2rx,2ry,2rz,-|r|^2) per reference point.
  - PE: for each query tile (128 queries), u = q_aug^T @ r_aug = 2 q.r - |r|^2
    into PSUM via K=4 fp32 matmuls (argmax(u) == argmin(d2), |q|^2 shift dropped).
  - DVE: hierarchical max over 64-element chunks (tensor_reduce on PSUM),
    then max8 + max_index -> top-8 chunk ids per query.
  - Host: exact re-rank of the top chunks' candidates on CPU jax, replicating the
    reference's fp32 arithmetic (incl. first-occurrence tie-break).
"""

import sys

for _p in ("/root/.axon_site/_ro/trn_rl_repo", "/opt/trn_rl_repo"):
    if _p not in sys.path:
        sys.path.append(_p)

import numpy as np

import concourse.bass as bass
import concourse.mybir as mybir
import concourse.tile as tile
from concourse.bass_utils import run_bass_kernel_spmd

# ---- monkeypatch: split the TileContext tail drain into single-wait drains ----
# core_v3 walrus codegen rejects instructions carrying more than one sync-wait
# command; the stock tail drain waits on every proc's final tick in one
# instruction.
from concourse.vector_clock import ScopedClock, VectorClock
from concourse.tile_sem_assignment import N_PROCS


def _split_drain_and_barrier(self, tick_clock, wait_clock):
    gc = tick_clock.global_clock
    for p in range(N_PROCS):
        t = gc[p]
        if t > 0:
            d = self.nc.sync.drain()
            vc = VectorClock([t if i == p else 0 for i in range(N_PROCS)])
            wait_clock.add_sem_waits(d.ins, ScopedClock({None: vc}))
    self.nc.all_engine_barrier()
    assert self.sems is not None
    popped = self.nc._tile_sem_poison_stack.pop()
    assert popped is self._sem_poison
    self.nc.clear_and_free_semaphores(list(self.sems.allocated().values()))
    self.nc.all_engine_barrier()


tile.TileContext._drain_and_barrier = _split_drain_and_barrier
# ------------------------------------------------------------------------------

import re


_PE_TICK_RE = re.compile(r"^PE_\d+$")


def _strip_same_engine_waits(nc):
    """Remove PE-tick sem waits from PE matmul instructions.

    Matmuls complete in pc order on TRN2 (monotone start AND end), so a
    matmul's wait on the PE tick semaphore (emitted by Tile for PSUM slot-reuse
    WAW) is always satisfied by program order. Stripping it keeps matmuls at
    one sync-wait, which is all core_v3 codegen accepts on the LW path.
    Other engines' same-engine waits are left alone (DVE's are load-bearing
    after Max/MaxIndex).
    """
    for name, inst in nc.inst_map.items():
        si = inst.sync_info
        if not si or not si.on_wait:
            continue
        if inst.engine != mybir.EngineType.PE:
            continue
        keep = [
            w
            for w in si.on_wait
            if not (
                w.sync_type == "semaphore"
                and w.ant_name is not None
                and _PE_TICK_RE.match(w.ant_name)
            )
        ]
        if len(keep) != len(si.on_wait):
            si.on_wait = keep

L1 = 4096   # reference points per batch
L2 = 4096   # query points per batch
N = 8       # batches == cores
C = 3
P = 128            # queries per tile (psum partition dim)
KAUG = 4           # contraction rows: qx,qy,qz,1
NCHUNK = 64        # hierarchy chunk width
NCH = L1 // NCHUNK           # 64 chunks
NTILES = L2 // P             # 32 query tiles
PSUM_FREE = 1024             # psum tile free dim (2 banks)
MM_FREE = 512                # matmul free dim (1 bank)
TOPC = 4                     # chunks re-ranked on host

_nc_cache = None


def _build_nc():
    nc = bass.Bass("TRN2", target_bir_lowering=False, debug=False, num_devices=N)
    qrT = nc.dram_tensor(
        "qrT", [KAUG, L2 + L1], mybir.dt.float32, kind="ExternalInput"
    ).ap()
    cids = nc.dram_tensor(
        "cids", [P, NTILES * 8], mybir.dt.uint32, kind="ExternalOutput"
    ).ap()

    with tile.TileContext(nc) as tc:
        with (
            tc.tile_pool(name="persist", bufs=1) as persist,
            tc.tile_pool(name="cmpool", bufs=2) as cmpool,
            tc.tile_pool(name="m8pool", bufs=2) as m8pool,
            tc.tile_pool(name="ps", bufs=4, space="PSUM") as ps,
        ):
            qr_sb = persist.tile([KAUG, L2 + L1], mybir.dt.float32)
            nc.sync.dma_start(qr_sb, qrT)
            q_sb = qr_sb[:, :L2]
            r_sb = qr_sb[:, L2:]

            cid_acc = persist.tile([P, NTILES, 8], mybir.dt.uint32)

            for t in range(NTILES):
                cm = cmpool.tile([P, NCH], mybir.dt.float32)
                lhsT = q_sb[:, t * P:(t + 1) * P]
                for h in range(L1 // PSUM_FREE):
                    psum = ps.tile([P, PSUM_FREE], mybir.dt.float32)
                    for j in range(PSUM_FREE // MM_FREE):
                        off = h * PSUM_FREE + j * MM_FREE
                        nc.tensor.matmul(
                            psum[:, j * MM_FREE:(j + 1) * MM_FREE],
                            lhsT=lhsT,
                            rhs=r_sb[:, off:off + MM_FREE],
                            start=True,
                            stop=True,
                        )
                    ncc = PSUM_FREE // NCHUNK
                    nc.vector.tensor_reduce(
                        cm[:, h * ncc:(h + 1) * ncc],
                        psum.rearrange("p (c k) -> p c k", k=NCHUNK),
                        axis=mybir.AxisListType.X,
                        op=mybir.AluOpType.max,
                    )
                m8 = m8pool.tile([P, 8], mybir.dt.float32)
                nc.vector.max(out=m8, in_=cm)
                nc.vector.max_index(out=cid_acc[:, t, :], in_max=m8, in_values=cm)

            nc.sync.dma_start(cids, cid_acc.rearrange("p t k -> p (t k)"))
    _strip_same_engine_waits(nc)
    return nc


def _get_nc():
    global _nc_cache
    if _nc_cache is None:
        _nc_cache = _build_nc()
    return _nc_cache


def _host_prep(coords1, coords2):
    """Build per-core qrT [4, L2+L1] fp32 arrays."""
    in_maps = []
    for n in range(N):
        q = coords2[:, n, :].astype(np.float32)   # [L2, C] queries
        r = coords1[:, n, :].astype(np.float32)   # [L1, C] refs
        qa = np.empty((KAUG, L2), np.float32)
        qa[0:3] = q.T
        qa[3] = 1.0
        ra = np.empty((KAUG, L1), np.float32)
        ra[0:3] = (2.0 * r).T
        rn = (r * r).sum(axis=1, dtype=np.float32)
        ra[3] = -rn
        in_maps.append({"qrT": np.concatenate([qa, ra], axis=1)})
    return in_maps


def _rerank(coords1, coords2, cand_chunks):
    """Exact fp32 re-rank of candidate chunks, replicating the reference formula
    on CPU jax. cand_chunks: [N, L2, TOPC] chunk ids. Returns nearest [N, L2]."""
    import jax
    import jax.numpy as jnp

    cpu = jax.devices("cpu")[0]
    nearest = np.empty((N, L2), np.int32)
    with jax.default_device(cpu):
        for n in range(N):
            q = jax.device_put(coords2[:, n, :].astype(np.float32), cpu)  # [L2, C]
            r = jax.device_put(coords1[:, n, :].astype(np.float32), cpu)  # [L1, C]
            t1 = jnp.sum(q * q, axis=-1)          # [L2]
            t2 = jnp.sum(r * r, axis=-1)          # [L1]
            ch = jax.device_put(cand_chunks[n].astype(np.int32), cpu)     # [L2, TOPC]
            # candidate indices [L2, TOPC*NCHUNK]
            cand = (ch[:, :, None] * NCHUNK
                    + jnp.arange(NCHUNK, dtype=jnp.int32)[None, None, :]
                    ).reshape(L2, TOPC * NCHUNK)
            rc = r[cand]                           # [L2, K, C]
            dots = jnp.einsum("lc,lkc->lk", q, rc)
            d2c = t1[:, None] + t2[cand] - 2.0 * dots
            d2c = np.asarray(d2c)
            cand = np.asarray(cand)
            dmin = d2c.min(axis=1, keepdims=True)
            # first-occurrence tie-break: smallest candidate index among ties
            masked = np.where(d2c == dmin, cand, np.iinfo(np.int32).max)
            nearest[n] = masked.min(axis=1).astype(np.int32)
    return nearest


def kernel(coords1, coords2):
    coords1 = np.asarray(coords1, dtype=np.float32)
    coords2 = np.asarray(coords2, dtype=np.float32)
    assert coords1.shape == (L1, N, C) and coords2.shape == (L2, N, C)

    in_maps = _host_prep(coords1, coords2)
    nc = _get_nc()
    res = run_bass_kernel_spmd(nc, in_maps, core_ids=list(range(N)))

    # cids[p, t*8+k] = k-th best chunk for query (t*128+p) of batch=core
    cand_chunks = np.empty((N, L2, TOPC), np.int32)
    for n in range(N):
        cids = res.results[n]["cids"].reshape(P, NTILES, 8)
        # query l = t*P + p
        cand_chunks[n] = cids[:, :, :TOPC].transpose(1, 0, 2).reshape(L2, TOPC)

    nearest = _rerank(coords1, coords2, cand_chunks)   # [N, L2]

    clusters = nearest.T.reshape(-1).astype(np.int32)
    batch_idx = np.broadcast_to(
        np.arange(N, dtype=np.int32), (L2, N)
    ).reshape(-1).copy()
    return clusters, batch_idx


if __name__ == "__main__":
    rng = np.random.default_rng(0)
    c1 = rng.random((L1, N, C), dtype=np.float32)
    c2 = rng.random((L2, N, C), dtype=np.float32)
    out = kernel(c1, c2)
    print("ok", out[0].shape, out[0].dtype, out[1].shape)


# revision 8
# speedup vs baseline: 2.6283x; 2.6283x over previous
"""Per-batch brute-force 1-NN (nearest cluster) on 8 Trainium2 cores.

Problem: coords1 [L1=4096, N=8, C=3] (reference points), coords2 [L2=4096, N=8, C=3]
(query points). For each batch n and query l, find argmin_m ||q - r||^2 within the
batch. Output: (clusters [L2*N] int32, batch_idx [L2*N] int32), matching
   nearest = argmin(d2, axis=-1) [N, L2]; clusters = nearest.T.reshape(-1)
   batch_idx = broadcast(arange(N), (L2, N)).reshape(-1)

Sharding: batch n -> core n (data parallel, no cross-core communication).

Device kernel (per core, one batch):
  - Host packs qrT [4, L2+L1] fp32: cols 0..L2-1 = (qx,qy,qz,1) per query,
    cols L2.. = (2rx,2ry,2rz,-|r|^2) per reference point.
  - PE: for each query tile (128 queries), u = q_aug^T @ r_aug = 2 q.r - |r|^2
    into PSUM via K=4 fp32 matmuls (argmax(u) == argmin(d2), |q|^2 shift dropped).
  - DVE: hierarchical max over 64-element chunks (tensor_reduce on PSUM),
    then max8 + max_index -> top-8 chunk ids per query.
  - Host: exact re-rank of the top chunks' candidates on CPU jax, replicating the
    reference's fp32 arithmetic (incl. first-occurrence tie-break).
"""

import sys

for _p in ("/root/.axon_site/_ro/trn_rl_repo", "/opt/trn_rl_repo"):
    if _p not in sys.path:
        sys.path.append(_p)

import numpy as np

import concourse.bass as bass
import concourse.mybir as mybir
import concourse.tile as tile
from concourse.bass_utils import run_bass_kernel_spmd

# ---- monkeypatch: split the TileContext tail drain into single-wait drains ----
# core_v3 walrus codegen rejects instructions carrying more than one sync-wait
# command; the stock tail drain waits on every proc's final tick in one
# instruction.
from concourse.vector_clock import ScopedClock, VectorClock
from concourse.tile_sem_assignment import N_PROCS


def _split_drain_and_barrier(self, tick_clock, wait_clock):
    gc = tick_clock.global_clock
    for p in range(N_PROCS):
        t = gc[p]
        if t > 0:
            d = self.nc.sync.drain()
            vc = VectorClock([t if i == p else 0 for i in range(N_PROCS)])
            wait_clock.add_sem_waits(d.ins, ScopedClock({None: vc}))
    self.nc.all_engine_barrier()
    assert self.sems is not None
    popped = self.nc._tile_sem_poison_stack.pop()
    assert popped is self._sem_poison
    self.nc.clear_and_free_semaphores(list(self.sems.allocated().values()))
    self.nc.all_engine_barrier()


tile.TileContext._drain_and_barrier = _split_drain_and_barrier
# ------------------------------------------------------------------------------

import re


def _strip_same_engine_waits(nc):
    """Remove same-engine engine-tick sem waits from PE/DVE compute ops.

    Engines execute and complete their compute instructions in issue order
    (PE matmuls are pc-monotone in start and end; DVE is a strict-FIFO
    pipeline with a DRAIN between ops), so a compute op's wait on its own
    engine tick semaphore (emitted by Tile for SBUF/PSUM slot-reuse hazards)
    is always satisfied by program order. Stripping keeps every instruction
    at one sync-wait, which is all core_v3 codegen accepts. Sequencer-sem
    waits are load-bearing (EventSemaphore barriers run out-of-band on the
    sequencer) and are never touched.
    """
    for name, inst in nc.inst_map.items():
        si = inst.sync_info
        if not si or not si.on_wait:
            continue
        ename = getattr(inst.engine, "name", str(inst.engine).split(".")[-1])
        if ename not in ("PE",):
            continue
        pat = re.compile(rf"^{ename}_\d+$")
        keep = [
            w
            for w in si.on_wait
            if not (
                w.sync_type == "semaphore"
                and w.ant_name is not None
                and pat.match(w.ant_name)
            )
        ]
        if len(keep) != len(si.on_wait):
            si.on_wait = keep

L1 = 4096   # reference points per batch
L2 = 4096   # query points per batch
N = 8       # batches == cores
C = 3
P = 128            # queries per tile (psum partition dim)
KAUG = 21          # contraction rows: bf16x3 limb-product pairs + |r|^2 limbs
NCHUNK = 64        # hierarchy chunk width
NCH = L1 // NCHUNK           # 64 chunks
NTILES = L2 // P             # 32 query tiles
PSUM_FREE = 2048             # psum tile free dim (4 banks)
MM_FREE = 512                # matmul free dim (1 bank)
TOPC = 6                     # chunks re-ranked on host

_nc_cache = None


def _build_nc():
    nc = bass.Bass("TRN2", target_bir_lowering=False, debug=False, num_devices=N)
    qrT = nc.dram_tensor(
        "qrT", [KAUG, L2 + L1], mybir.dt.bfloat16, kind="ExternalInput"
    ).ap()
    cids = nc.dram_tensor(
        "cids", [P, NTILES * 8], mybir.dt.uint32, kind="ExternalOutput"
    ).ap()

    with tile.TileContext(nc) as tc:
        with (
            tc.tile_pool(name="persist", bufs=1) as persist,
            tc.tile_pool(name="cmpool", bufs=NTILES) as cmpool,
            tc.tile_pool(name="m8pool", bufs=NTILES) as m8pool,
            tc.tile_pool(name="ps", bufs=2, space="PSUM") as ps,
        ):
            qr_sb = persist.tile([KAUG, L2 + L1], mybir.dt.bfloat16)
            nc.sync.dma_start(qr_sb, qrT)
            q_sb = qr_sb[:, :L2]
            r_sb = qr_sb[:, L2:]

            cid_acc = persist.tile([P, NTILES, 8], mybir.dt.uint32)

            for t in range(NTILES):
                cm = cmpool.tile([P, NCH], mybir.dt.float32)
                lhsT = q_sb[:, t * P:(t + 1) * P]
                for h in range(L1 // PSUM_FREE):
                    psum = ps.tile([P, PSUM_FREE], mybir.dt.float32)
                    for j in range(PSUM_FREE // MM_FREE):
                        off = h * PSUM_FREE + j * MM_FREE
                        nc.tensor.matmul(
                            psum[:, j * MM_FREE:(j + 1) * MM_FREE],
                            lhsT=lhsT,
                            rhs=r_sb[:, off:off + MM_FREE],
                            start=True,
                            stop=True,
                        )
                    ncc = PSUM_FREE // NCHUNK
                    nc.vector.tensor_reduce(
                        cm[:, h * ncc:(h + 1) * ncc],
                        psum.rearrange("p (c k) -> p c k", k=NCHUNK),
                        axis=mybir.AxisListType.X,
                        op=mybir.AluOpType.max,
                    )
                m8 = m8pool.tile([P, 8], mybir.dt.float32)
                nc.vector.max(out=m8, in_=cm)
                nc.vector.max_index(out=cid_acc[:, t, :], in_max=m8, in_values=cm)

            # Funnel through one more DVE op before the DMA: MaxIndex's sem-inc
            # can fire before its SBUF writeback fully drains, and the output
            # DMA otherwise races the last tile's write. The copy's own DRAIN
            # (same-engine pipeline flush) orders the read after the write.
            cid_out = persist.tile([P, NTILES, 8], mybir.dt.uint32)
            nc.vector.tensor_copy(cid_out, cid_acc)
            nc.sync.dma_start(cids, cid_out.rearrange("p t k -> p (t k)"))
    _strip_same_engine_waits(nc)
    return nc


def _get_nc():
    global _nc_cache
    if _nc_cache is None:
        _nc_cache = _build_nc()
    return _nc_cache


import ml_dtypes

_BF16 = ml_dtypes.bfloat16


def _split3(x):
    """fp32 -> three bf16 limbs (as fp32 values): x ~= h + m + l."""
    h = x.astype(_BF16).astype(np.float32)
    r1 = (x - h).astype(np.float32)
    m = r1.astype(_BF16).astype(np.float32)
    l = (r1 - m).astype(np.float32).astype(_BF16).astype(np.float32)
    return h, m, l


def _host_prep(coords1, coords2):
    """Build per-core qrT [21, L2+L1] bf16 arrays.

    u = 2 q.r - |r|^2 is computed on-device as a single K=21 bf16 matmul:
    q and 2r are split into 3 bf16 limbs each; the 6 significant limb-product
    pairs per coordinate (hh, hm, mh, hl, lh, mm) plus the 3 limbs of -|r|^2
    (weight 1) form the contraction rows. bf16*bf16 products are exact in the
    fp32 PSUM accumulation, so u matches fp32 arithmetic to ~2e-6.
    """
    in_maps = []
    for n in range(N):
        q = coords2[:, n, :].astype(np.float32)   # [L2, C] queries
        r = coords1[:, n, :].astype(np.float32)   # [L1, C] refs
        qh, qm, ql = _split3(q)
        rh, rm, rl = _split3((2.0 * r).astype(np.float32))
        rn = -(r * r).sum(axis=1, dtype=np.float32)
        nh, nm, nl = _split3(rn)
        Wr, Sr = [], []
        for c in range(C):
            for w, s in ((qh, rh), (qh, rm), (qm, rh), (qh, rl), (ql, rh), (qm, rm)):
                Wr.append(w[:, c])
                Sr.append(s[:, c])
        one = np.ones(L2, np.float32)
        for s in (nh, nm, nl):
            Wr.append(one)
            Sr.append(s)
        qa = np.stack(Wr).astype(_BF16)   # [21, L2]
        ra = np.stack(Sr).astype(_BF16)   # [21, L1]
        in_maps.append({"qrT": np.concatenate([qa, ra], axis=1)})
    return in_maps


def _rerank(coords1, coords2, cand_chunks):
    """Exact fp32 re-rank of candidate chunks, replicating the reference formula
    on CPU jax. cand_chunks: [N, L2, TOPC] chunk ids. Returns nearest [N, L2]."""
    import jax
    import jax.numpy as jnp

    cpu = jax.devices("cpu")[0]
    nearest = np.empty((N, L2), np.int32)
    with jax.default_device(cpu):
        for n in range(N):
            q = jax.device_put(coords2[:, n, :].astype(np.float32), cpu)  # [L2, C]
            r = jax.device_put(coords1[:, n, :].astype(np.float32), cpu)  # [L1, C]
            t1 = jnp.sum(q * q, axis=-1)          # [L2]
            t2 = jnp.sum(r * r, axis=-1)          # [L1]
            ch = jax.device_put(cand_chunks[n].astype(np.int32), cpu)     # [L2, TOPC]
            # candidate indices [L2, TOPC*NCHUNK]
            cand = (ch[:, :, None] * NCHUNK
                    + jnp.arange(NCHUNK, dtype=jnp.int32)[None, None, :]
                    ).reshape(L2, TOPC * NCHUNK)
            rc = r[cand]                           # [L2, K, C]
            dots = jnp.einsum("lc,lkc->lk", q, rc)
            d2c = t1[:, None] + t2[cand] - 2.0 * dots
            d2c = np.asarray(d2c)
            cand = np.asarray(cand)
            dmin = d2c.min(axis=1, keepdims=True)
            # first-occurrence tie-break: smallest candidate index among ties
            masked = np.where(d2c == dmin, cand, np.iinfo(np.int32).max)
            nearest[n] = masked.min(axis=1).astype(np.int32)
    return nearest


def kernel(coords1, coords2):
    coords1 = np.asarray(coords1, dtype=np.float32)
    coords2 = np.asarray(coords2, dtype=np.float32)
    assert coords1.shape == (L1, N, C) and coords2.shape == (L2, N, C)

    in_maps = _host_prep(coords1, coords2)
    nc = _get_nc()
    res = run_bass_kernel_spmd(nc, in_maps, core_ids=list(range(N)))

    # cids[p, t*8+k] = k-th best chunk for query (t*128+p) of batch=core
    cand_chunks = np.empty((N, L2, TOPC), np.int32)
    for n in range(N):
        cids = res.results[n]["cids"].reshape(P, NTILES, 8)
        # query l = t*P + p
        cand_chunks[n] = cids[:, :, :TOPC].transpose(1, 0, 2).reshape(L2, TOPC)

    nearest = _rerank(coords1, coords2, cand_chunks)   # [N, L2]

    clusters = nearest.T.reshape(-1).astype(np.int32)
    batch_idx = np.broadcast_to(
        np.arange(N, dtype=np.int32), (L2, N)
    ).reshape(-1).copy()
    return clusters, batch_idx


if __name__ == "__main__":
    rng = np.random.default_rng(0)
    c1 = rng.random((L1, N, C), dtype=np.float32)
    c2 = rng.random((L2, N, C), dtype=np.float32)
    out = kernel(c1, c2)
    print("ok", out[0].shape, out[0].dtype, out[1].shape)


# revision 10
# speedup vs baseline: 3.0162x; 1.1476x over previous
"""Per-batch brute-force 1-NN (nearest cluster) on 8 Trainium2 cores.

Problem: coords1 [L1=4096, N=8, C=3] (reference points), coords2 [L2=4096, N=8, C=3]
(query points). For each batch n and query l, find argmin_m ||q - r||^2 within the
batch. Output: (clusters [L2*N] int32, batch_idx [L2*N] int32), matching
   nearest = argmin(d2, axis=-1) [N, L2]; clusters = nearest.T.reshape(-1)
   batch_idx = broadcast(arange(N), (L2, N)).reshape(-1)

Sharding: batch n -> core n (data parallel, no cross-core communication).

Device pipeline (per core, one batch, per 128-query tile):
  - PE: one K=24 bf16 matmul per 512 refs computes u = 2 q.r - |r|^2 - |q|^2
    (= -d^2) into PSUM fp32: q, 2r, -|r|^2, -|q|^2 are each split into 3 bf16
    limbs and the significant limb-product pairs form the contraction rows;
    bf16*bf16 products are exact in fp32, so u matches fp32 arithmetic ~2e-6.
  - ACT: evacuates PSUM -> SBUF fp16 (u in [-3, 0), top values ~ -d2_min, so
    fp16 quantization near the top is ~1e-6).
  - DVE: log2 pairwise in-place tensor_tensor(max) folds on fp16 (2 elem/cycle)
    down to 64 strided-chunk maxima, then max8 + max_index -> top-8 chunk ids.
    Chunk k = indices {k + 64*i}.
  - Host: exact re-rank of the top TOPC chunks' candidates on CPU jax,
    replicating the reference's fp32 arithmetic (incl. first-occurrence ties).
"""

import sys

for _p in ("/root/.axon_site/_ro/trn_rl_repo", "/opt/trn_rl_repo"):
    if _p not in sys.path:
        sys.path.append(_p)

import re

import ml_dtypes
import numpy as np

import concourse.bass as bass
import concourse.mybir as mybir
import concourse.tile as tile
from concourse.bass_utils import run_bass_kernel_spmd

# ---- monkeypatch: split the TileContext tail drain into single-wait drains ----
# core_v3 walrus codegen rejects instructions carrying more than one sync-wait
# command; the stock tail drain waits on every proc's final tick in one
# instruction.
from concourse.vector_clock import ScopedClock, VectorClock
from concourse.tile_sem_assignment import N_PROCS


def _split_drain_and_barrier(self, tick_clock, wait_clock):
    gc = tick_clock.global_clock
    for p in range(N_PROCS):
        t = gc[p]
        if t > 0:
            d = self.nc.sync.drain()
            vc = VectorClock([t if i == p else 0 for i in range(N_PROCS)])
            wait_clock.add_sem_waits(d.ins, ScopedClock({None: vc}))
    self.nc.all_engine_barrier()
    assert self.sems is not None
    popped = self.nc._tile_sem_poison_stack.pop()
    assert popped is self._sem_poison
    self.nc.clear_and_free_semaphores(list(self.sems.allocated().values()))
    self.nc.all_engine_barrier()


tile.TileContext._drain_and_barrier = _split_drain_and_barrier
# ------------------------------------------------------------------------------


def _strip_same_engine_waits(nc):
    """Remove PE-tick sem waits from PE matmul instructions.

    Matmuls complete in pc order on TRN2 (monotone start AND end), so a
    matmul's wait on the PE tick semaphore (emitted by Tile for PSUM slot-reuse
    WAW) is always satisfied by program order. Stripping keeps matmuls at one
    sync-wait, which is all core_v3 codegen accepts on the LW path.
    """
    for name, inst in nc.inst_map.items():
        si = inst.sync_info
        if not si or not si.on_wait:
            continue
        ename = getattr(inst.engine, "name", str(inst.engine).split(".")[-1])
        if ename not in ("PE", "Activation"):
            continue
        pat = re.compile(rf"^{ename}_\d+$")
        keep = [
            w
            for w in si.on_wait
            if not (
                w.sync_type == "semaphore"
                and w.ant_name is not None
                and pat.match(w.ant_name)
            )
        ]
        if len(keep) != len(si.on_wait):
            si.on_wait = keep


L1 = 4096   # reference points per batch
L2 = 4096   # query points per batch
N = 8       # batches == cores
C = 3
P = 128            # queries per tile (psum partition dim)
KAUG = 24          # contraction rows (bf16 limb-product pairs + |r|^2, |q|^2 limbs)
NCHUNK = 64        # number of strided positions per chunk
NCH = L1 // NCHUNK           # 64 chunks (chunk k = {k + 64*i})
NTILES = L2 // P             # 32 query tiles
PSUM_FREE = 2048             # psum tile free dim (4 banks)
MM_FREE = 512                # matmul free dim (1 bank)
EVBUFS = 3                   # fp16 evacuation tiles in flight
TOPC = 6                     # chunks re-ranked on host

_nc_cache = None


def _build_nc():
    nc = bass.Bass("TRN2", target_bir_lowering=False, debug=False, num_devices=N)
    qrT = nc.dram_tensor(
        "qrT", [KAUG, L2 + L1], mybir.dt.bfloat16, kind="ExternalInput"
    ).ap()
    cids = nc.dram_tensor(
        "cids", [P, NTILES * 8], mybir.dt.uint32, kind="ExternalOutput"
    ).ap()

    with tile.TileContext(nc) as tc:
        with (
            tc.tile_pool(name="persist", bufs=1) as persist,
            tc.tile_pool(name="evpool", bufs=EVBUFS) as evpool,
            tc.tile_pool(name="cmpool", bufs=NTILES) as cmpool,
            tc.tile_pool(name="m8pool", bufs=NTILES) as m8pool,
            tc.tile_pool(name="junkpool", bufs=NTILES) as junkpool,
            tc.tile_pool(name="ps", bufs=2, space="PSUM") as ps,
        ):
            qr_sb = persist.tile([KAUG, L2 + L1], mybir.dt.bfloat16)
            nc.sync.dma_start(qr_sb, qrT)
            q_sb = qr_sb[:, :L2]
            r_sb = qr_sb[:, L2:]

            cid_acc = persist.tile([P, NTILES, 8], mybir.dt.uint32)
            cm_tiles = []

            for t in range(NTILES):
                ev = evpool.tile([P, L1], mybir.dt.float16)
                cm = cmpool.tile([P, NCH], mybir.dt.float32)
                cm_tiles.append(cm)
                lhsT = q_sb[:, t * P:(t + 1) * P]

                if t >= EVBUFS:
                    # Absorb the ev-slot WAR (DVE fold of tile t-EVBUFS must
                    # finish) into a tiny dedicated ACT op, so the evacuation
                    # activations below carry only their PE wait (codegen
                    # accepts a single sync-wait per instruction).
                    junk = junkpool.tile([1, 1], mybir.dt.float32)
                    nc.scalar.activation(
                        out=junk,
                        in_=cm_tiles[t - EVBUFS][0:1, 0:1],
                        func=mybir.ActivationFunctionType.Copy,
                    )

                for h in range(L1 // PSUM_FREE):
                    psum = ps.tile([P, PSUM_FREE], mybir.dt.float32)
                    for j in range(PSUM_FREE // MM_FREE):
                        off = h * PSUM_FREE + j * MM_FREE
                        nc.tensor.matmul(
                            psum[:, j * MM_FREE:(j + 1) * MM_FREE],
                            lhsT=lhsT,
                            rhs=r_sb[:, off:off + MM_FREE],
                            start=True,
                            stop=True,
                        )
                    nc.scalar.activation(
                        out=ev[:, h * PSUM_FREE:(h + 1) * PSUM_FREE],
                        in_=psum,
                        func=mybir.ActivationFunctionType.Copy,
                    )

                # in-place pairwise max folds: 4096 -> 128 (fp16, 2x mode)
                w = L1 // 2
                while w >= NCH * 2:
                    nc.vector.tensor_tensor(
                        ev[:, :w], ev[:, :w], ev[:, w:2 * w], mybir.AluOpType.max
                    )
                    w //= 2
                # final fold 128 -> 64, cast to fp32 cm
                nc.vector.tensor_tensor(
                    cm, ev[:, :NCH], ev[:, NCH:2 * NCH], mybir.AluOpType.max
                )

                m8 = m8pool.tile([P, 8], mybir.dt.float32)
                nc.vector.max(out=m8, in_=cm)
                nc.vector.max_index(out=cid_acc[:, t, :], in_max=m8, in_values=cm)

            # Funnel through one more DVE op before the DMA: MaxIndex's sem-inc
            # can fire before its SBUF writeback fully drains, and the output
            # DMA otherwise races the last tile's write. The copy's own DRAIN
            # orders the read after the write.
            cid_out = persist.tile([P, NTILES, 8], mybir.dt.uint32)
            nc.vector.tensor_copy(cid_out, cid_acc)
            nc.sync.dma_start(cids, cid_out.rearrange("p t k -> p (t k)"))
    _strip_same_engine_waits(nc)
    return nc


def _get_nc():
    global _nc_cache
    if _nc_cache is None:
        _nc_cache = _build_nc()
    return _nc_cache


_BF16 = ml_dtypes.bfloat16


def _split3(x):
    """fp32 -> three bf16 limbs (as fp32 values): x ~= h + m + l."""
    h = x.astype(_BF16).astype(np.float32)
    r1 = (x - h).astype(np.float32)
    m = r1.astype(_BF16).astype(np.float32)
    l = (r1 - m).astype(np.float32).astype(_BF16).astype(np.float32)
    return h, m, l


def _host_prep(coords1, coords2):
    """Build per-core qrT [24, L2+L1] bf16 arrays (see module docstring)."""
    in_maps = []
    for n in range(N):
        q = coords2[:, n, :].astype(np.float32)   # [L2, C] queries
        r = coords1[:, n, :].astype(np.float32)   # [L1, C] refs
        qh, qm, ql = _split3(q)
        rh, rm, rl = _split3((2.0 * r).astype(np.float32))
        rn = -(r * r).sum(axis=1, dtype=np.float32)       # -|r|^2
        nh, nm, nl = _split3(rn)
        t1 = (q * q).sum(axis=1, dtype=np.float32)        # |q|^2
        th, tm, tl = _split3(t1)
        Wr, Sr = [], []
        for c in range(C):
            for w, s in ((qh, rh), (qh, rm), (qm, rh), (qh, rl), (ql, rh), (qm, rm)):
                Wr.append(w[:, c])
                Sr.append(s[:, c])
        one = np.ones(L2, np.float32)
        negone = np.full(L1, -1.0, np.float32)
        for s in (nh, nm, nl):      # + (-|r|^2) via weight 1
            Wr.append(one)
            Sr.append(s)
        for w in (th, tm, tl):      # + (-|q|^2) via stream -1
            Wr.append(w)
            Sr.append(negone)
        qa = np.stack(Wr).astype(_BF16)   # [24, L2]
        ra = np.stack(Sr).astype(_BF16)   # [24, L1]
        in_maps.append({"qrT": np.concatenate([qa, ra], axis=1)})
    return in_maps


def _rerank(coords1, coords2, cand_chunks):
    """Exact fp32 re-rank of candidate chunks, replicating the reference formula
    on CPU jax. cand_chunks: [N, L2, TOPC] strided-chunk ids (chunk k =
    {k + 64*i}). Returns nearest [N, L2]."""
    import jax
    import jax.numpy as jnp

    cpu = jax.devices("cpu")[0]
    nearest = np.empty((N, L2), np.int32)
    with jax.default_device(cpu):
        for n in range(N):
            q = jax.device_put(coords2[:, n, :].astype(np.float32), cpu)  # [L2, C]
            r = jax.device_put(coords1[:, n, :].astype(np.float32), cpu)  # [L1, C]
            t1 = jnp.sum(q * q, axis=-1)          # [L2]
            t2 = jnp.sum(r * r, axis=-1)          # [L1]
            ch = jax.device_put(cand_chunks[n].astype(np.int32), cpu)     # [L2, TOPC]
            # candidate indices [L2, TOPC*NCHUNK]: chunk k covers {k + 64*i}
            cand = (ch[:, :, None]
                    + NCH * jnp.arange(NCHUNK, dtype=jnp.int32)[None, None, :]
                    ).reshape(L2, TOPC * NCHUNK)
            rc = r[cand]                           # [L2, K, C]
            dots = jnp.einsum("lc,lkc->lk", q, rc)
            d2c = t1[:, None] + t2[cand] - 2.0 * dots
            d2c = np.asarray(d2c)
            cand = np.asarray(cand)
            dmin = d2c.min(axis=1, keepdims=True)
            # first-occurrence tie-break: smallest candidate index among ties
            masked = np.where(d2c == dmin, cand, np.iinfo(np.int32).max)
            nearest[n] = masked.min(axis=1).astype(np.int32)
    return nearest


def kernel(coords1, coords2):
    coords1 = np.asarray(coords1, dtype=np.float32)
    coords2 = np.asarray(coords2, dtype=np.float32)
    assert coords1.shape == (L1, N, C) and coords2.shape == (L2, N, C)

    in_maps = _host_prep(coords1, coords2)
    nc = _get_nc()
    res = run_bass_kernel_spmd(nc, in_maps, core_ids=list(range(N)))

    # cids[p, t*8+k] = k-th best strided chunk for query (t*128+p) of batch=core
    cand_chunks = np.empty((N, L2, TOPC), np.int32)
    for n in range(N):
        cids = res.results[n]["cids"].reshape(P, NTILES, 8)
        cand_chunks[n] = cids[:, :, :TOPC].transpose(1, 0, 2).reshape(L2, TOPC)

    nearest = _rerank(coords1, coords2, cand_chunks)   # [N, L2]

    clusters = nearest.T.reshape(-1).astype(np.int32)
    batch_idx = np.broadcast_to(
        np.arange(N, dtype=np.int32), (L2, N)
    ).reshape(-1).copy()
    return clusters, batch_idx


if __name__ == "__main__":
    rng = np.random.default_rng(0)
    c1 = rng.random((L1, N, C), dtype=np.float32)
    c2 = rng.random((L2, N, C), dtype=np.float32)
    out = kernel(c1, c2)
    print("ok", out[0].shape, out[0].dtype, out[1].shape)
